# revision 49
# baseline (speedup 1.0000x reference)
"""Causal single-head attention (B=8, S=2048, D=1024, fp32) on 8 NeuronCores.

Data-parallel over batch: one batch element per core, weights replicated.
All matmuls in bf16 (PE weight loads fully pipelined -> ~N*0.417ns/matmul),
fp32 PSUM accumulation, fully SBUF-resident (no DRAM scratch).

Host pre-casts x and W to bf16 (round-to-nearest-even) so the device wire
format is 2-byte: 10MB in / 4MB out per core, no on-device weight casts.

Per-core pipeline:
  1. xT = x.T via PE transposes (bf16, 1 cycle/row)
  2. V  = x @ Wv    -> vsb [ki, kt, e] bf16   (direct SBUF layout)
     QT = Wq.T @ xT -> qt  [e, et, s]  bf16
     KT = Wk.T @ xT -> kt  [e, et, s]  bf16
  3. per 512-wide query chunk c, k-tile k<=4c+3:
       S^T[k,c] accumulated over 8 e-tiles; exp(s/32) on ScalarE -> est bf16
       diagonal tiles multiplied by 0/1 causal mask on GpSimd
  4. per 128-row q tile: rowsum + AV via est-weight-reuse matmul groups
     (rs N=2 self-loading, then 2x N=512 with ldweights=False), normalize
     on VectorE (bf16 out), DMA out. Host widens to fp32.
"""

import numpy as np

B, S, D = 8, 2048, 1024
P = 128
NCORES = 8

_built = None


def _bf16_bits(a):
    """fp32 ndarray -> uint16 bf16 bits, round-to-nearest-even."""
    b = np.ascontiguousarray(a, dtype=np.float32).view(np.uint32)
    r = b + 0x7FFF + ((b >> 16) & 1)
    return (r >> 16).astype(np.uint16)


def _build():
    import concourse.tile as tile
    import concourse.mybir as mybir
    from concourse import bacc

    FP32 = mybir.dt.float32
    BF16 = mybir.dt.bfloat16
    U16 = mybir.dt.uint16
    AF = mybir.ActivationFunctionType

    nc = bacc.Bacc("TRN2", target_bir_lowering=False, debug=False, num_devices=NCORES)
    x_d = nc.dram_tensor("x16", [S, D], U16, kind="ExternalInput").ap().bitcast(BF16)
    wq_d = nc.dram_tensor("Wq16", [D, D], U16, kind="ExternalInput").ap().bitcast(BF16)
    wk_d = nc.dram_tensor("Wk16", [D, D], U16, kind="ExternalInput").ap().bitcast(BF16)
    wv_d = nc.dram_tensor("Wv16", [D, D], U16, kind="ExternalInput").ap().bitcast(BF16)
    out_d = nc.dram_tensor("out", [S, D], U16, kind="ExternalOutput").ap().bitcast(BF16)

    ident_c = nc.inline_tensor(
        (np.eye(P) * 0x3F80).astype(np.uint16), name="ident_c"
    )
    # sliding causal 0/1 mask in bf16 bits: tile j uses cols [(3-j)*128, +512);
    # value at (p, y) = 1 iff y >= p + 384 else 0. cols 896:898 = ones.
    zz = np.arange(896)[None, :]
    pp = np.arange(P)[:, None]
    m01 = np.where(zz >= pp + 384, 0x3F80, 0x0000).astype(np.uint16)
    m01 = np.concatenate([m01, np.full((P, 2), 0x3F80, np.uint16)], axis=1)
    mask_c = nc.inline_tensor(m01, name="mask_c")

    with tile.TileContext(nc) as tc:
        with (
            tc.tile_pool(name="xtp", bufs=1) as xtp,
            tc.tile_pool(name="qtp", bufs=1) as qtp,
            tc.tile_pool(name="ktp", bufs=1) as ktp,
            tc.tile_pool(name="vbp", bufs=1) as vbp,
            tc.tile_pool(name="xsp", bufs=6) as xsp,
            tc.tile_pool(name="wbp", bufs=16) as wbp,
            tc.tile_pool(name="estp", bufs=64) as estp,
            tc.tile_pool(name="osbp", bufs=2) as osbp,
            tc.tile_pool(name="smp", bufs=1) as smp,
            tc.tile_pool(name="rcpp", bufs=2) as rcpp,
            tc.tile_pool(name="ps", bufs=8, space="PSUM") as ps,
        ):
            ident = smp.tile([P, P], BF16, tag="ident")
            nc.sync.dma_start(out=ident, in_=ident_c.ap().bitcast(BF16))
            mask = smp.tile([P, 898], BF16, tag="mask")
            ones2 = mask[:, 896:898]
            # memset (no DMA dependency) operand for the p-state warmup
            # matmuls; results never read
            junk = smp.tile([P, 640], BF16, tag="junk")
            nc.vector.memset(junk, 0.0)

            xt = xtp.tile([P, 8, S], BF16, tag="xt")
            qt = qtp.tile([P, 8, S], BF16, tag="qt")
            kt = ktp.tile([P, 8, S], BF16, tag="kt")
            vsb = vbp.tile([P, 16, D], BF16, tag="vsb")

            # PE p-state warmup on the memset tile (no DMA dependency, so the
            # PE ramps from t~=8us while the preamble DMAs stream in).
            for r in range(14):
                dps = ps.tile([P, 512], FP32, tag="ps", name=f"warm{r}")
                nc.tensor.matmul(
                    dps, lhsT=junk[:, 0:P], rhs=junk[:, 128:640],
                    start=True, stop=True,
                )

            x_pend = {}

            def load_x(si, engs=(nc.sync,)):
                x_tile = xsp.tile([P, D], BF16, tag="xs", name=f"x{si}")
                engs[si % len(engs)].dma_start(
                    out=x_tile, in_=x_d[si * P:(si + 1) * P, :]
                )
                x_pend[si] = x_tile

            def load_w(w_d, wname, engs=(nc.sync,)):
                wb = []
                for kd in range(8):
                    w_t = wbp.tile([P, D], BF16, tag="wb", name=f"{wname}{kd}")
                    engs[kd % len(engs)].dma_start(
                        out=w_t, in_=w_d[kd * P:(kd + 1) * P, :]
                    )
                    wb.append(w_t)
                return wb

            ncopy = 0

            def copy_cast(out, in_):
                nonlocal ncopy
                eng = (nc.vector.tensor_copy, nc.scalar.copy)[ncopy % 2]
                eng(out=out, in_=in_)
                ncopy += 1

            # ---- phase A: transpose group g (4 s-tiles -> xt columns) ----
            def transpose_group(g):
                xts = [x_pend.pop(si) for si in range(4 * g, 4 * g + 4)]
                for kd in range(8):
                    tp4 = ps.tile([P, 512], BF16, tag="ps", name=f"tp{g}_{kd}")
                    for j in range(4):
                        nc.tensor.matmul(
                            tp4[:, j * P:(j + 1) * P],
                            lhsT=xts[j][:, kd * P:(kd + 1) * P],
                            rhs=ident,
                            is_transpose=True,
                            start=(j == 0),
                            stop=(j == 3),
                        )
                    copy_cast(out=xt[:, kd, g * 512:(g + 1) * 512], in_=tp4)

            # ---- phase B: V projection for group g ----
            def v_proj_group(g, wvb):
                for st_i in range(4 * g, 4 * g + 4):
                    for ec in range(2):
                        pst = ps.tile([P, 512], FP32, tag="ps", name=f"v{st_i}_{ec}")
                        for kd in range(8):
                            nc.tensor.matmul(
                                pst,
                                lhsT=xt[:, kd, st_i * P:(st_i + 1) * P],
                                rhs=wvb[kd][:, ec * 512:(ec + 1) * 512],
                                start=(kd == 0),
                                stop=(kd == 7),
                            )
                        copy_cast(
                            out=vsb[:, st_i, ec * 512:(ec + 1) * 512], in_=pst
                        )

            # ---- phase C/D: Q^T / K^T projection, sc-outer ----
            def qk_proj_sc(sc, wb, dst):
                for et in range(8):
                    pst = ps.tile([P, 512], FP32, tag="ps", name=f"p{sc}_{et}")
                    for kd in range(8):
                        nc.tensor.matmul(
                            pst,
                            lhsT=wb[kd][:, et * P:(et + 1) * P],
                            rhs=xt[:, kd, sc * 512:(sc + 1) * 512],
                            start=(kd == 0),
                            stop=(kd == 7),
                        )
                    copy_cast(out=dst[:, et, sc * 512:(sc + 1) * 512], in_=pst)

            # ---- phase E: S^T + exp for 128-wide q tile c1 (exact causal) ----
            est_tiles = {}

            def s_stage(c1):
                for k in range(c1 + 1):
                    sps = ps.tile([P, P], FP32, tag="ps", name=f"s{c1}_{k}")
                    for e in range(8):
                        nc.tensor.matmul(
                            sps,
                            lhsT=kt[:, e, k * P:(k + 1) * P],
                            rhs=qt[:, e, c1 * P:(c1 + 1) * P],
                            start=(e == 0),
                            stop=(e == 7),
                        )
                    est = estp.tile([P, P], BF16, tag="est", name=f"e{c1}_{k}")
                    nc.scalar.activation(out=est, in_=sps, func=AF.Exp, scale=0.03125)
                    if k == c1:
                        nc.vector.tensor_mul(
                            est, est, mask[:, 384:384 + P]
                        )
                    est_tiles[(c1, k)] = est

            # ---- phase F: rowsum + AV + normalize for chunk c ----
            def av_stage(c):
                for j in range(4):
                    q_abs = 4 * c + j
                    rs = ps.tile([P, 2], FP32, tag="ps", name=f"rs{q_abs}")
                    o0 = ps.tile([P, 512], FP32, tag="ps", name=f"o0_{q_abs}")
                    o1 = ps.tile([P, 512], FP32, tag="ps", name=f"o1_{q_abs}")
                    for k in range(q_abs + 1):
                        lhs = est_tiles[(q_abs, k)]
                        st = (k == 0)
                        sp = (k == q_abs)
                        nc.tensor.matmul(
                            o0, lhsT=lhs, rhs=vsb[:, k, 0:512], start=st, stop=sp
                        )
                        m2 = nc.tensor.matmul(
                            o1, lhsT=lhs, rhs=vsb[:, k, 512:1024], start=st, stop=sp
                        )
                        m2.ins.ldweights = False
                        m3 = nc.tensor.matmul(rs, lhsT=lhs, rhs=ones2, start=st, stop=sp)
                        m3.ins.ldweights = False
                    rec = rcpp.tile([P, 1], FP32, tag="rcp", name=f"rc{q_abs}")
                    nc.vector.reciprocal(rec, rs[:, 0:1])
                    o_sb = osbp.tile([P, D], BF16, tag="osb", name=f"ob{q_abs}")
                    nc.vector.tensor_scalar_mul(o_sb[:, 0:512], o0, rec)
                    nc.sync.dma_start(
                        out=out_d[q_abs * P:(q_abs + 1) * P, 0:512],
                        in_=o_sb[:, 0:512],
                    )
                    nc.vector.tensor_scalar_mul(o_sb[:, 512:1024], o1, rec)
                    nc.sync.dma_start(
                        out=out_d[q_abs * P:(q_abs + 1) * P, 512:1024],
                        in_=o_sb[:, 512:1024],
                    )

            # ---- orchestration ----
            for si in range(4):
                load_x(si)
            wvb = load_w(wv_d, "wv")
            # mask consts are first needed in phase E (~190us in): load them
            # behind the x group-0 / Wv preamble so they don't delay it
            nc.sync.dma_start(out=mask, in_=mask_c.ap().bitcast(BF16))
            transpose_group(0)
            for si in range(4, 8):
                load_x(si)
            v_proj_group(0, wvb)
            transpose_group(1)
            for si in range(8, 12):
                load_x(si)
            v_proj_group(1, wvb)
            wqb = load_w(wq_d, "wq")
            transpose_group(2)
            for si in range(12, 16):
                load_x(si)
            v_proj_group(2, wvb)
            transpose_group(3)
            v_proj_group(3, wvb)
            wkb = load_w(wk_d, "wk")
            for sc in range(4):
                qk_proj_sc(sc, wqb, qt)
            for sc in range(4):
                qk_proj_sc(sc, wkb, kt)
            for c in range(4):
                for c1 in range(4 * c, 4 * c + 4):
                    s_stage(c1)
                av_stage(c)

    nc.compile()
    return nc


def _get_nc():
    global _built
    if _built is None:
        _built = _build()
    return _built


def _run(inputs, trace=False):
    from concourse.bass_utils import run_bass_kernel_spmd

    x = np.asarray(inputs["x"])
    wq16 = _bf16_bits(inputs["Wq"])
    wk16 = _bf16_bits(inputs["Wk"])
    wv16 = _bf16_bits(inputs["Wv"])
    in_maps = [
        {
            "x16": _bf16_bits(x[c]),
            "Wq16": wq16,
            "Wk16": wk16,
            "Wv16": wv16,
        }
        for c in range(NCORES)
    ]
    res = run_bass_kernel_spmd(
        nc=_get_nc(), in_maps=in_maps, core_ids=list(range(NCORES)), trace=trace
    )
    out = np.stack(
        [
            (
                np.asarray(res.results[c]["out"])
                .view(np.uint16)
                .astype(np.uint32)
                << 16
            ).view(np.float32)
            for c in range(NCORES)
        ],
        axis=0,
    )
    return out, res


def kernel(x, Wq, Wk, Wv):
    out, _ = _run({"x": x, "Wq": Wq, "Wk": Wk, "Wv": Wv}, trace=False)
    return out
